# revision 9
# baseline (speedup 1.0000x reference)
"""VQ codebook lookup (nn_Codebook) on 8 Trainium2 NeuronCores.

Data-parallel: 32768 tokens sharded 4096/core (core c -> batch c//2,
THW-half c%2); the [2048, 256] codebook is replicated.

Scores need fp32-grade exactness (min top-2 gap on this problem size is
~1e-5, and a flipped argmin swaps an entire 256-wide codebook row in the
output).  The PE's fp32 path is 4 cyc/row and fp32r is only an 8-bit
mantissa format, so scores are computed with an fp16 pair-split:
x = xh + xl, e = eh + el (fp16 each, split exact to 2^-24), and
x.e ~= xh.eh + xh.el + xl.eh accumulated in fp32 PSUM (fp16 products are
exact in fp32; dropped xl.el term < 1e-5, verified 0 argmin flips).

Per-core device pipeline (32 token-tiles of 128):
  PE   : 3-term fp16 score matmuls, [128,512] banks, K=256 as 2 chunks.
  DVE  : tensor_tensor_reduce fuses bias add (-0.5||e||^2, replicated
         [128,2048] from host), PSUM->SBUF evict, and per-token running
         max (chained across the two [128,1024] PSUM halves).
  DVE  : scalar_tensor_tensor (scores >= smax) * iota, sum-accum ->
         argmax index (2x fp32 SBUF mode).
  Pool : indirect DMA gathers the 128 winning codebook rows per tile.
  PE   : transposes gathered [tok,C] tiles to [C,tok] via identity matmul.
  ACT  : PSUM->SBUF copies of the transposed tiles.
Host finishes the two scalars: commitment loss from
0.25*(sum(z^2) - 2*sum(smax))/numel (sum(z^2) is input-only, like the
bias), and perplexity from a bincount of the returned indices.
"""

import numpy as np

import concourse.bass as bass
import concourse.bacc as bacc
import concourse.mybir as mybir
import concourse.tile as tile
from concourse import bass_utils

F32 = mybir.dt.float32
F16 = mybir.dt.float16
U32 = mybir.dt.uint32
ALU = mybir.AluOpType

B, C, T, H, W = 4, 256, 8, 32, 32
THW = T * H * W              # 8192
N_CODES, D = 2048, 256
N_CORES = 8
TPC = B * THW // N_CORES     # 4096 tokens per core
NT = TPC // 128              # 32 token tiles per core
NEG_INF = -3.0e38


def _build_kernel():
    nc = bacc.Bacc(
        "TRN2",
        target_bir_lowering=False,
        debug=False,
        enable_asserts=False,
        num_devices=N_CORES,
    )

    zh_d = nc.dram_tensor("z_hi", [2, 128, TPC], F16, kind="ExternalInput")
    zl_d = nc.dram_tensor("z_lo", [2, 128, TPC], F16, kind="ExternalInput")
    eh_d = nc.dram_tensor("embT_hi", [2, 128, N_CODES], F16, kind="ExternalInput")
    el_d = nc.dram_tensor("embT_lo", [2, 128, N_CODES], F16, kind="ExternalInput")
    emb_rows_d = nc.dram_tensor("emb_rows", [N_CODES, D], F32, kind="ExternalInput")
    bias_d = nc.dram_tensor("bias_rep", [128, N_CODES], F32, kind="ExternalInput")
    iota_d = nc.dram_tensor("iota_rep", [128, N_CODES], F32, kind="ExternalInput")
    eye_d = nc.dram_tensor("eye128", [128, 128], F32, kind="ExternalInput")

    out_d = nc.dram_tensor("emb_out", [2, 128, TPC], F32, kind="ExternalOutput")
    idx_d = nc.dram_tensor("idx_out", [128, NT], U32, kind="ExternalOutput")
    smax_d = nc.dram_tensor("smax_out", [128, NT], F32, kind="ExternalOutput")

    with tile.TileContext(nc) as tc:
        with (
            tc.tile_pool(name="big", bufs=1) as big,
            tc.tile_pool(name="sc", bufs=2) as scp,
            tc.tile_pool(name="ind", bufs=2) as indp,
            tc.tile_pool(name="gat", bufs=6) as gatp,
            tc.tile_pool(name="ps", bufs=3, space="PSUM") as psp,
            tc.tile_pool(name="tp", bufs=2, space="PSUM") as tpp,
        ):
            zh0 = big.tile([128, TPC], F16)
            zh1 = big.tile([128, TPC], F16)
            zl0 = big.tile([128, TPC], F16)
            zl1 = big.tile([128, TPC], F16)
            eh0 = big.tile([128, N_CODES], F16)
            eh1 = big.tile([128, N_CODES], F16)
            el0 = big.tile([128, N_CODES], F16)
            el1 = big.tile([128, N_CODES], F16)
            bias_sb = big.tile([128, N_CODES], F32)
            iota_sb = big.tile([128, N_CODES], F32)
            eye_sb = big.tile([128, 128], F32)
            embout0 = big.tile([128, TPC], F32)
            embout1 = big.tile([128, TPC], F32)
            smax_sb = big.tile([128, NT], F32)
            idxf_sb = big.tile([128, NT], F32)
            idxu_sb = big.tile([128, NT], U32)

            nc.sync.dma_start(eh0[:], eh_d.ap()[0])
            nc.sync.dma_start(eh1[:], eh_d.ap()[1])
            nc.sync.dma_start(el0[:], el_d.ap()[0])
            nc.sync.dma_start(el1[:], el_d.ap()[1])
            # z loads chunked so tile-0 matmuls start after the first 1/4
            for q in range(4):
                qs = bass.ts(q, TPC // 4)
                nc.sync.dma_start(zh0[:, qs], zh_d.ap()[0][:, qs])
                nc.sync.dma_start(zh1[:, qs], zh_d.ap()[1][:, qs])
                nc.sync.dma_start(zl0[:, qs], zl_d.ap()[0][:, qs])
                nc.sync.dma_start(zl1[:, qs], zl_d.ap()[1][:, qs])
            nc.sync.dma_start(bias_sb[:], bias_d.ap())
            nc.sync.dma_start(iota_sb[:], iota_d.ap())
            nc.sync.dma_start(eye_sb[:], eye_d.ap())

            LAG = 3
            gathered = []

            def emit_transpose(j):
                jtok = bass.ts(j, 128)
                gj = gathered[j]
                for h in range(2):
                    gt = tpp.tile([128, 128], F32, tag="gt", name=f"gt_{j}_{h}")
                    nc.tensor.transpose(gt[:], gj[:, bass.ts(h, 128)], eye_sb[:])
                    nc.scalar.copy((embout0 if h == 0 else embout1)[:, jtok], gt[:])
                if j % 8 == 7:
                    chunk = bass.ts(j // 8, 1024)
                    nc.sync.dma_start(out_d.ap()[0][:, chunk], embout0[:, chunk])
                    nc.sync.dma_start(out_d.ap()[1][:, chunk], embout1[:, chunk])

            for i in range(NT):
                tok = bass.ts(i, 128)
                scores = scp.tile([128, N_CODES], F32, tag="scores")
                for h in range(2):
                    ps_h = psp.tile([128, 1024], F32, tag="ps_h")
                    for n in range(2):
                        cod = bass.ts(2 * h + n, 512)
                        dst = ps_h[:, bass.ts(n, 512)]
                        # x.e ~= xh.eh + xh.el + xl.eh, K=256 in 2 chunks
                        terms = [
                            (zh0, eh0, zh1, eh1),
                            (zh0, el0, zh1, el1),
                            (zl0, eh0, zl1, eh1),
                        ]
                        for t, (a0, b0, a1, b1) in enumerate(terms):
                            nc.tensor.matmul(
                                dst, a0[:, tok], b0[:, cod],
                                start=(t == 0), stop=False,
                            )
                            nc.tensor.matmul(
                                dst, a1[:, tok], b1[:, cod],
                                start=False, stop=(t == 2),
                            )
                    # bias add + PSUM->SBUF evict (InstTensorScalarPtr)
                    nc.vector.scalar_tensor_tensor(
                        out=scores[:, bass.ts(h, 1024)],
                        in0=ps_h[:],
                        scalar=0.0,
                        in1=bias_sb[:, bass.ts(h, 1024)],
                        op0=ALU.bypass,
                        op1=ALU.add,
                    )
                nc.vector.reduce_max(
                    smax_sb[:, i:i + 1], scores[:], axis=mybir.AxisListType.X
                )
                # index extraction: sum((score >= smax) * iota)
                ind = indp.tile([128, N_CODES], F32, tag="ind")
                nc.vector.scalar_tensor_tensor(
                    out=ind[:],
                    in0=scores[:],
                    scalar=smax_sb[:, i:i + 1],
                    in1=iota_sb[:],
                    op0=ALU.is_ge,
                    op1=ALU.mult,
                    accum_out=idxf_sb[:, i:i + 1],
                )
                nc.vector.tensor_copy(idxu_sb[:, i:i + 1], idxf_sb[:, i:i + 1])

                g = gatp.tile([128, D], F32, tag="g")
                gathered.append(g)
                nc.gpsimd.indirect_dma_start(
                    out=g[:],
                    out_offset=None,
                    in_=emb_rows_d.ap(),
                    in_offset=bass.IndirectOffsetOnAxis(
                        ap=idxu_sb[:, i:i + 1], axis=0
                    ),
                    bounds_check=N_CODES - 1,
                    oob_is_err=False,
                )
                # Transposes lag 3 tiles behind so their gather dependency has
                # cleared by the time the in-order PE queue reaches them.
                if i >= LAG:
                    emit_transpose(i - LAG)

            for j in range(NT - LAG, NT):
                emit_transpose(j)

            nc.sync.dma_start(idx_d.ap(), idxu_sb[:])
            nc.sync.dma_start(smax_d.ap(), smax_sb[:])

    nc.compile()
    return nc


_NC_CACHE = {}


def _get_nc():
    if "nc" not in _NC_CACHE:
        _NC_CACHE["nc"] = _build_kernel()
    return _NC_CACHE["nc"]


def _make_in_maps(z, embeddings):
    z = np.ascontiguousarray(np.asarray(z, dtype=np.float32))
    emb = np.ascontiguousarray(np.asarray(embeddings, dtype=np.float32))
    zr = z.reshape(B, C, THW)

    z_hi = zr.astype(np.float16)
    z_lo = (zr - z_hi.astype(np.float32)).astype(np.float16)
    embT_full = np.ascontiguousarray(emb.T)                  # [256, 2048]
    eT_hi = embT_full.astype(np.float16)
    eT_lo = (embT_full - eT_hi.astype(np.float32)).astype(np.float16)
    eh = np.stack([eT_hi[:128], eT_hi[128:]])
    el = np.stack([eT_lo[:128], eT_lo[128:]])

    bias_row = (-0.5 * (emb.astype(np.float64) ** 2).sum(axis=1)).astype(np.float32)
    bias_rep = np.ascontiguousarray(np.broadcast_to(bias_row, (128, N_CODES)))
    iota_rep = np.ascontiguousarray(
        np.broadcast_to(np.arange(N_CODES, dtype=np.float32), (128, N_CODES))
    )
    eye = np.eye(128, dtype=np.float32)

    in_maps = []
    for c in range(N_CORES):
        b, half = c // 2, c % 2
        sl = slice(half * TPC, (half + 1) * TPC)
        in_maps.append({
            "z_hi": np.ascontiguousarray(
                np.stack([z_hi[b, :128, sl], z_hi[b, 128:, sl]])),
            "z_lo": np.ascontiguousarray(
                np.stack([z_lo[b, :128, sl], z_lo[b, 128:, sl]])),
            "embT_hi": eh,
            "embT_lo": el,
            "emb_rows": emb,
            "bias_rep": bias_rep,
            "iota_rep": iota_rep,
            "eye128": eye,
        })
    return in_maps, float((zr.astype(np.float64) ** 2).sum())


def _unshard(results, zsq_total):
    emb_full = np.empty((B, C, THW), dtype=np.float32)
    idx_full = np.empty((B, THW), dtype=np.int64)
    smax_total = 0.0
    for c in range(N_CORES):
        b, half = c // 2, c % 2
        sl = slice(half * TPC, (half + 1) * TPC)
        r = results[c]
        emb_full[b, :128, sl] = r["emb_out"][0]
        emb_full[b, 128:, sl] = r["emb_out"][1]
        # idx_out[p, i] is token (i*128 + p) of this half
        idx_full[b, sl] = r["idx_out"].astype(np.int64).T.reshape(-1)
        smax_total += r["smax_out"].astype(np.float64).sum()

    n_tok = B * THW
    loss = np.float32(0.25 * (zsq_total - 2.0 * smax_total) / (n_tok * D))
    counts = np.bincount(idx_full.ravel(), minlength=N_CODES).astype(np.float64)
    avg = counts / n_tok
    perplexity = np.float32(np.exp(-np.sum(avg * np.log(avg + 1e-10))))

    emb_st = emb_full.reshape(B, C, T, H, W)
    enc_idx = idx_full.reshape(B, T, H, W).astype(np.int32)
    return emb_st, enc_idx, loss, perplexity


def kernel(z, embeddings):
    nc = _get_nc()
    in_maps, zsq_total = _make_in_maps(z, embeddings)
    res = bass_utils.run_bass_kernel_spmd(nc, in_maps, core_ids=list(range(N_CORES)))
    return _unshard(res.results, zsq_total)
